# revision 1
# baseline (speedup 1.0000x reference)
"""Causal multi-head attention (B=4, H=16, S=2048, D=64) on 8 TRN2 NeuronCores.

Sharding: B*H = 64 (batch, head) pairs -> 8 per core, fully independent,
no collectives.

Per-core algorithm (per head):
  - Load Q, K natural layout, cast f32->bf16 during DMA (SWDGE).
  - PE-transpose Q, K into [64=d, 2048=s] layout (contraction dim on
    partitions).
  - For each k-block kb (128 keys): S^T[kb] = K[kb] @ Q^T computed as
    matmul(lhsT=KT[:, kb], rhs=QT[:, q>=kb*128]) -> PSUM [128, q], then
    exp(0.125 * S^T) on ScalarE -> U^T[kb] (bf16, unnormalized probs,
    transposed). Diagonal block masked by upper-triangular multiply.
  - For each q-block qb: O[qb] = sum_kb U^T[kb][:, qb].T @ [V[kb] | 1]
    accumulated in PSUM [128, 65]; column 64 is the softmax denominator.
    Normalize with per-partition reciprocal multiply, store f32.
"""

import numpy as np

import concourse.bass as bass
import concourse.tile as tile
from concourse import mybir
from concourse.bass_utils import run_bass_kernel_spmd
from concourse.masks import make_identity, make_upper_triangular
from concourse.vector_clock import ScopedClock, VectorClock

F32 = mybir.dt.float32
BF16 = mybir.dt.bfloat16

B, H, S, D = 4, 16, 2048, 64
N_CORES = 8
HEADS_PER_CORE = B * H // N_CORES  # 8
NB = S // 128  # 16 blocks of 128
SCALE = 1.0 / np.sqrt(np.float32(D))  # 0.125


def _patch_tile_drain():
    """This walrus build rejects >1 sem wait on the kernel-tail Drain
    instruction ("Too many sync wait commands"). Spread the waits across
    single-wait NOPs on the sync engine instead."""
    if getattr(tile.TileContext, "_drain_patched", False):
        return

    def _drain_and_barrier(self, tick_clock, wait_clock):
        gc = tick_clock.global_clock
        n = len(gc)
        for i in range(n):
            if gc[i] > 0:
                vc = VectorClock([gc[j] if j == i else 0 for j in range(n)])
                nop_inst = self.nc.sync.nop(nofuse=True, hint=f"drainwait{i}")
                wait_clock.add_sem_waits(nop_inst.ins, ScopedClock({None: vc}))
        self.nc.sync.drain()
        self.nc.all_engine_barrier()
        popped = self.nc._tile_sem_poison_stack.pop()
        assert popped is self._sem_poison
        self.nc.clear_and_free_semaphores(list(self.sems.allocated().values()))
        self.nc.all_engine_barrier()

    tile.TileContext._drain_and_barrier = _drain_and_barrier
    tile.TileContext._drain_patched = True


_patch_tile_drain()


def _split_multi_waits(nc, limit=1):
    """This walrus build allows at most one sem wait per instruction.
    Move excess waits onto same-engine NOPs inserted just before."""
    ctr = [0]
    for func in nc.m.functions:
        for bb in func.blocks:
            insts = list(bb.instructions)
            out = []
            changed = False
            for inst in insts:
                si = inst.sync_info
                if si is not None and si.on_wait is not None and len(si.on_wait) > limit:
                    waits = list(si.on_wait)
                    extra, keep = waits[:-limit], waits[-limit:]
                    for w in extra:
                        ctr[0] += 1
                        nop = mybir.InstNoOp(
                            name=f"waitsplit-{ctr[0]}", ins=[], outs=[]
                        )
                        nop.engine = inst.engine
                        nop.sync_info = mybir.SyncInfo(on_wait=[w], on_update=[])
                        out.append(nop)
                    inst.sync_info = mybir.SyncInfo(
                        on_wait=keep, on_update=list(si.on_update or [])
                    )
                    changed = True
                out.append(inst)
            if changed:
                try:
                    bb.instructions[:] = out
                except Exception:
                    bb.instructions = out
    return nc


def build_nc(n_heads: int = HEADS_PER_CORE):
    nc = bass.Bass("TRN2", target_bir_lowering=False)
    q_d = nc.dram_tensor("queries", [n_heads, S, D], F32, kind="ExternalInput")
    k_d = nc.dram_tensor("keys", [n_heads, S, D], F32, kind="ExternalInput")
    v_d = nc.dram_tensor("values", [n_heads, S, D], F32, kind="ExternalInput")
    o_d = nc.dram_tensor("out", [n_heads, S, D], F32, kind="ExternalOutput")

    # [h, p, n, d] views: s = n*128 + p
    v_r = v_d[:].rearrange("h (n p) d -> h p n d", p=128)
    o_r = o_d[:].rearrange("h (n p) d -> h p n d", p=128)

    with tile.TileContext(nc) as tc:
        with (
            tc.tile_pool(name="const", bufs=1) as constp,
            tc.tile_pool(name="scr", bufs=4, space="DRAM") as scrp,
            tc.tile_pool(name="tp", bufs=4) as tpp,
            tc.tile_pool(name="vpool", bufs=4) as vpp,
            tc.tile_pool(name="ut", bufs=3) as utp,
            tc.tile_pool(name="oh", bufs=3) as ohp,
            tc.tile_pool(name="rz", bufs=4) as rzp,
            tc.tile_pool(name="ps_s", bufs=3, space="PSUM") as ps_s,
            tc.tile_pool(name="ps_o", bufs=2, space="PSUM") as ps_o,
        ):
            trimask = constp.tile([128, 128], BF16)
            make_upper_triangular(nc, trimask, val=1.0, diag=True)
            zpad = constp.tile([128, 1024], BF16)
            nc.vector.memset(zpad, 0.0)

            # All heads' Q/K casts into DRAM scratch are issued up front (no
            # WAR hazards between them), so the per-head cast->transpose
            # latency chain (~10-20us) is paid once, not per head at startup.
            # Transposes and V loads are software-pipelined PIPE heads ahead
            # of compute. Stores go on the sync HWDGE ring: SWDGE lanes then
            # carry only prompt-completing load DMAs, so the cumulative
            # lane-sem waits on transposes never couple to a store.
            PIPE = 2  # transpose/V lookahead (heads)
            CAST_AHEAD = 2  # scratch-cast lookahead (heads)
            scrs = {}
            vps = {}
            loaded = {}

            def issue_casts(h):
                # [S, 128] scratch: cast writes cols 0-63, cols 64-127 are
                # zeroed, so the transposed [128, S] operands have zeros in
                # rows 64-127 (128-row weight shape keeps the PE weight path
                # double-buffered; zero rows contract harmlessly).
                scrq = scrp.tile([S, 128], BF16, tag="scrq")
                scrk = scrp.tile([S, 128], BF16, tag="scrk")
                nc.gpsimd.dma_start(out=scrq[:, D : 2 * D], in_=zpad)
                nc.gpsimd.dma_start(out=scrk[:, D : 2 * D], in_=zpad)
                nc.gpsimd.dma_start(out=scrq[:, 0:D], in_=q_d[h])
                nc.gpsimd.dma_start(out=scrk[:, 0:D], in_=k_d[h])
                scrs[h] = (scrq, scrk)

            def issue_v(h):
                vp = vpp.tile([128, NB, D + 1], BF16, tag="vp")
                nc.gpsimd.dma_start(out=vp[:, :, 0:D], in_=v_r[h])
                vps[h] = vp

            def issue_xbar(h):
                scrq, scrk = scrs.pop(h)
                qt = tpp.tile([128, S], BF16, tag="qt")
                kt = tpp.tile([128, S], BF16, tag="kt")
                nc.sync.dma_start(out=qt, in_=scrq[:, :], transpose=True)
                nc.sync.dma_start(out=kt, in_=scrk[:, :], transpose=True)
                loaded[h] = (qt, kt)

            for h in range(min(PIPE, n_heads)):
                issue_v(h)
            for h in range(min(CAST_AHEAD, n_heads)):
                issue_casts(h)
            for h in range(min(PIPE, n_heads)):
                issue_xbar(h)

            for h in range(n_heads):
                if h + CAST_AHEAD < n_heads:
                    issue_casts(h + CAST_AHEAD)
                if h + PIPE < n_heads:
                    issue_v(h + PIPE)
                    issue_xbar(h + PIPE)
                qt, kt = loaded.pop(h)
                vp = vps.pop(h)
                nc.vector.memset(vp[:, :, D : D + 1], 1.0)

                # --- scores + exp, per k-block ---
                uts = []
                for kb in range(NB):
                    L = S - kb * 128  # valid q length (q >= kb*128)
                    ut = utp.tile([128, L], BF16, tag=f"ut{kb}")
                    uts.append(ut)
                    off = 0
                    while off < L:
                        tl = min(1024, L - off)
                        ps = ps_s.tile([128, 1024], F32, tag="s")
                        for c0 in range(0, tl, 512):
                            cl = min(512, tl - c0)
                            nc.tensor.matmul(
                                ps[:, c0 : c0 + cl],
                                lhsT=kt[:, kb * 128 : (kb + 1) * 128],
                                rhs=qt[:, kb * 128 + off + c0 : kb * 128 + off + c0 + cl],
                                start=True,
                                stop=True,
                            )
                        nc.scalar.activation(
                            out=ut[:, off : off + tl],
                            in_=ps[:, 0:tl],
                            func=mybir.ActivationFunctionType.Exp,
                            scale=float(SCALE),
                        )
                        off += tl
                    # mask diagonal block: keep k <= q  (partition <= free)
                    nc.vector.tensor_mul(ut[:, 0:128], ut[:, 0:128], trimask)

                # --- O = P @ [V | 1], per q-block ---
                oh = ohp.tile([128, NB, D], F32, tag="oh")
                for qb in range(NB):
                    po = ps_o.tile([128, D + 1], F32, tag="o")
                    for kb in range(qb + 1):
                        nc.tensor.matmul(
                            po,
                            lhsT=uts[kb][:, (qb - kb) * 128 : (qb - kb) * 128 + 128],
                            rhs=vp[:, kb, :],
                            start=(kb == 0),
                            stop=(kb == qb),
                        )
                    rz = rzp.tile([128, 1], F32, tag="rz")
                    nc.vector.reciprocal(rz, po[:, D : D + 1])
                    nc.vector.tensor_scalar_mul(oh[:, qb, :], po[:, 0:D], rz)
                # Store via sync HWDGE: SWDGE lanes then carry only
                # load-class DMAs (prompt completion), so the cumulative
                # lane-sem waits on transposes never couple to a store.
                # The store's own wait blocks only transposes issued ~2 heads
                # later, which is harmless.
                nc.sync.dma_start(out=o_r[h], in_=oh)
    _split_multi_waits(nc)
    return nc


_NC_CACHE = {}


def _get_nc(n_heads: int = HEADS_PER_CORE):
    if n_heads not in _NC_CACHE:
        _NC_CACHE[n_heads] = build_nc(n_heads)
    return _NC_CACHE[n_heads]


def make_in_maps(queries, keys, values):
    qf = np.ascontiguousarray(
        np.asarray(queries, dtype=np.float32).reshape(B * H, S, D)
    )
    kf = np.ascontiguousarray(np.asarray(keys, dtype=np.float32).reshape(B * H, S, D))
    vf = np.ascontiguousarray(
        np.asarray(values, dtype=np.float32).reshape(B * H, S, D)
    )
    n = HEADS_PER_CORE
    return [
        {
            "queries": qf[i * n : (i + 1) * n],
            "keys": kf[i * n : (i + 1) * n],
            "values": vf[i * n : (i + 1) * n],
        }
        for i in range(N_CORES)
    ]


def kernel(keys, queries, values, head_dim=None, **_ignored):
    nc = _get_nc()
    in_maps = make_in_maps(queries, keys, values)
    res = run_bass_kernel_spmd(nc, in_maps, core_ids=list(range(N_CORES)))
    out = np.concatenate([res.results[i]["out"] for i in range(N_CORES)], axis=0)
    return out.reshape(B, H, S, D).astype(np.float32)

